# revision 1
# baseline (speedup 1.0000x reference)
"""MedianConvolution (gnn message passing) — Trainium2 Bass kernel, 8 cores.

Computes: h = x @ kernel; msg = h[neighbors]; out = exact midpoint median
over the K=32 neighbor axis (ranks 15,16 of the sort), i.e.
tfp percentile(q=50, interpolation='midpoint').

Distribution: nodes (rows of x's output / neighbors) are sharded across the
8 NeuronCores; every core computes the full h = x @ kernel on-device
(kernel/x replicated) and gathers only its own node shard's neighbor rows.

Per-core SPMD program:
  phase 1  GEMM: xT is supplied host-pre-transposed ([256, N]); PE computes
           h tile-by-tile (fp32, PSUM-accumulated over the two 128-feature
           chunks); h rows are written to DRAM, split into two halves
           (h_lo = rows [0, N/2), h_hi = rows [N/2, N)), each with one extra
           "+BIG" dummy row. The split exists because dma_gather indices are
           int16 (max 32767 < N); every neighbor is fetched by two gather
           calls (one per half, the miss side pointing at the dummy row) and
           merged with one elementwise min.
  phase 2  per chunk of C shard nodes: gpsimd.dma_gather pulls the 256-byte
           h rows for all 32 neighbor planes (k-major layout
           [128, K, C/128, 64]), a TT-min merges the lo/hi candidates, a
           Batcher odd-even mergesort sorts planes 0-15 and 16-31 (strided
           multi-dim APs, ping-pong between two buffers, untouched planes
           copied on the Scalar engine), and the 32-way median pair is
           extracted with the anti-diagonal identity
              low = max_i min(A_i, B_15-i),  up = min_i max(A_i, B_15-i)
           via two TT ops + two segmented tensor_reduce ops. The midpoint
           (low+up)/2 is DMAed out.
"""
from contextlib import ExitStack

import numpy as np

import concourse.bass as bass
import concourse.tile as tile
from concourse import bacc, bass_utils, library_config, mybir
from concourse.tile_rust import add_dep_helper

F32 = mybir.dt.float32
I16 = mybir.dt.int16
P = 128
U = 64  # units
K = 32  # neighbors
FEAT = 256
N_NODES = 50000
BIG = 1.0e30
NUM_CORES = 8
CHUNK = 256  # shard nodes per chunk
NET_BUFS = 4

# Batcher odd-even mergesort(16) stages; verified against np.sort via the
# 0-1 principle. Each stage: comparators (k, k+d) for k = i*f + r over the
# slices below, applied to both 16-plane halves. cp = untouched plane
# slices (copied to the ping-pong destination).
SORT16_STAGES = [
    dict(f=2, i=(0, 8, 1), r=(0, 1, 1), d=1, cp=[]),
    dict(f=4, i=(0, 4, 1), r=(0, 2, 1), d=2, cp=[]),
    dict(f=4, i=(0, 4, 1), r=(1, 2, 1), d=1, cp=[(0, 16, 4), (3, 16, 4)]),
    dict(f=8, i=(0, 2, 1), r=(0, 4, 1), d=4, cp=[]),
    dict(f=8, i=(0, 2, 1), r=(2, 4, 1), d=2,
         cp=[(0, 16, 8), (1, 16, 8), (6, 16, 8), (7, 16, 8)]),
    dict(f=8, i=(0, 2, 1), r=(1, 6, 2), d=1, cp=[(0, 16, 8), (7, 16, 8)]),
    dict(f=16, i=(0, 1, 1), r=(0, 8, 1), d=8, cp=[]),
    dict(f=16, i=(0, 1, 1), r=(4, 8, 1), d=4, cp=[(0, 4, 1), (12, 16, 1)]),
    dict(f=4, i=(0, 3, 1), r=(2, 4, 1), d=2, cp=[(0, 2, 1), (14, 16, 1)]),
    dict(f=2, i=(0, 7, 1), r=(1, 2, 1), d=1, cp=[(0, 16, 15)]),
]


def build_kernel(nrows, shard_nodes, C, num_cores=NUM_CORES, gemm_super=2048,
                 net_bufs=NET_BUFS):
    assert nrows % 2 == 0
    HALF = nrows // 2
    NCHUNK = shard_nodes // C
    assert NCHUNK * C == shard_nodes
    B = C // P
    NIDX = C * K
    IDXCOLS = NIDX // 16

    nc = bacc.Bacc(
        "TRN2",
        target_bir_lowering=False,
        debug=False,
        num_devices=num_cores,
    )

    xT = nc.dram_tensor("xT", [FEAT, nrows], F32, kind="ExternalInput").ap()
    wk = nc.dram_tensor("wk", [FEAT, U], F32, kind="ExternalInput").ap()
    idx = nc.dram_tensor("idx", [NCHUNK, 2, P, IDXCOLS], I16, kind="ExternalInput").ap()
    out = nc.dram_tensor("out", [NCHUNK, P, B * U], F32, kind="ExternalOutput").ap()
    h_lo = nc.dram_tensor("h_lo", [HALF + 1, U], F32, kind="Internal").ap()
    h_hi = nc.dram_tensor("h_hi", [HALF + 1, U], F32, kind="Internal").ap()

    with tile.TileContext(nc) as tc:
        with ExitStack() as ctx:
            # ---------------- phase 1: GEMM ----------------
            ctx1 = ctx.enter_context(ExitStack())
            g_x = ctx1.enter_context(tc.tile_pool(name="g_x", bufs=2))
            g_w = ctx1.enter_context(tc.tile_pool(name="g_w", bufs=1))
            g_h = ctx1.enter_context(tc.tile_pool(name="g_h", bufs=2))
            g_ps = ctx1.enter_context(tc.tile_pool(name="g_ps", bufs=4, space="PSUM"))

            wkt = g_w.tile([P, 2 * U], F32)
            nc.sync.dma_start(wkt[:, 0:U], wk[0:P, :])
            nc.sync.dma_start(wkt[:, U : 2 * U], wk[P : 2 * P, :])

            h_writes = []
            dummy = g_w.tile([1, U], F32)
            nc.vector.memset(dummy[:], BIG)
            h_writes.append(nc.sync.dma_start(h_lo[HALF : HALF + 1, :], dummy[:]))
            h_writes.append(nc.sync.dma_start(h_hi[HALF : HALF + 1, :], dummy[:]))

            S = gemm_super
            n_super = (nrows + S - 1) // S
            for s in range(n_super):
                n0 = s * S
                ncnt = min(S, nrows - n0)
                ntiles = (ncnt + P - 1) // P
                xt0 = g_x.tile([P, S], F32, tag="xt0")
                xt1 = g_x.tile([P, S], F32, tag="xt1")
                nc.sync.dma_start(xt0[:, 0:ncnt], xT[0:P, n0 : n0 + ncnt])
                nc.sync.dma_start(xt1[:, 0:ncnt], xT[P : 2 * P, n0 : n0 + ncnt])
                hb = g_h.tile([P, (S // P) * U], F32, tag="hb")
                for t in range(ntiles):
                    c0 = t * P
                    cw = min(P, ncnt - c0)
                    ps = g_ps.tile([P, U], F32)
                    nc.tensor.matmul(
                        ps[0:cw, :], xt0[:, c0 : c0 + cw], wkt[:, 0:U],
                        start=True, stop=False,
                    )
                    nc.tensor.matmul(
                        ps[0:cw, :], xt1[:, c0 : c0 + cw], wkt[:, U : 2 * U],
                        start=False, stop=True,
                    )
                    nc.scalar.copy(hb[0:cw, t * U : (t + 1) * U], ps[0:cw, :])
                hb3 = hb[:].rearrange("p (t u) -> p t u", u=U)
                # write h rows into the lo/hi half regions (straddle-aware,
                # full 128-row tiles coalesced into single DMAs)
                for lim0, lim1, dst, base in (
                    (n0, min(n0 + ncnt, HALF), h_lo, 0),
                    (max(n0, HALF), n0 + ncnt, h_hi, HALF),
                ):
                    if lim1 <= lim0:
                        continue
                    ta = (lim0 - n0 + P - 1) // P
                    tb = (lim1 - n0) // P
                    segs = []
                    if ta > tb:
                        segs.append((lim0, lim1))
                    else:
                        if lim0 < n0 + ta * P:
                            segs.append((lim0, n0 + ta * P))
                        if tb > ta:
                            segs.append((n0 + ta * P, n0 + tb * P))
                        if n0 + tb * P < lim1:
                            segs.append((n0 + tb * P, lim1))
                    for r0, r1 in segs:
                        nt = (r1 - r0) // P
                        if nt >= 1 and (r0 - n0) % P == 0:
                            tt = (r0 - n0) // P
                            dr = dst[r0 - base : r1 - base, :].rearrange(
                                "(o p) u -> p o u", p=P
                            )
                            srcv = hb3[:, tt : tt + nt, :]
                        else:
                            tt = (r0 - n0) // P
                            p0 = r0 - (n0 + tt * P)
                            p1 = r1 - (n0 + tt * P)
                            dr = dst[r0 - base : r1 - base, :].rearrange(
                                "(o p) u -> p o u", p=p1 - p0
                            )
                            srcv = hb3[p0:p1, tt : tt + 1, :]
                        h_writes.append(nc.sync.dma_start(dr, srcv))

            # ---------------- phase 2: gather + median ----------------
            ctx1.close()
            g_net = ctx.enter_context(tc.tile_pool(name="g_net", bufs=net_bufs))
            g_idx = ctx.enter_context(tc.tile_pool(name="g_idx", bufs=2))
            g_out = ctx.enter_context(tc.tile_pool(name="g_out", bufs=2))
            g_big = ctx.enter_context(tc.tile_pool(name="g_big", bufs=1))

            nc.gpsimd.load_library(library_config.mlp)
            med_all = g_big.tile([P, NCHUNK * B * U], F32, tag="medall")
            n_g = 0
            BU = B * U
            # per-call index count capped by the 128-entry SWDGE ring
            KG = max(1, 1920 // C)
            kgroups = []
            k0 = 0
            while k0 < K:
                kgroups.append((k0, min(K, k0 + KG)))
                k0 += KG

            for c in range(NCHUNK):
                ia = g_idx.tile([P, IDXCOLS], I16, tag="ia")
                ib = g_idx.tile([P, IDXCOLS], I16, tag="ib")
                nc.sync.dma_start(ia[:], idx[c, 0])
                nc.sync.dma_start(ib[:], idx[c, 1])
                ra = g_net.tile([P, K * BU], F32, tag="ra")
                rb = g_net.tile([P, K * BU], F32, tag="rb")
                for reg, it, hsrc in ((ra, ia, h_lo), (rb, ib, h_hi)):
                    for ka, kb in kgroups:
                        nidx = C * (kb - ka)
                        g = nc.gpsimd.dma_gather(
                            reg[:, ka * BU : kb * BU].rearrange("p (j e) -> p j e", e=U),
                            hsrc[:],
                            it[:, ka * C // 16 : kb * C // 16],
                            nidx,
                            nidx,
                            U,
                            single_packet=False,
                        )
                        if n_g == 0:
                            for w in h_writes:
                                add_dep_helper(
                                    g.ins, w.ins,
                                    reason="gather waits for h DRAM writes",
                                )
                        n_g += 1
                # merge lo/hi candidates (dummy rows are +BIG)
                nc.vector.tensor_tensor(
                    out=ra[:], in0=ra[:], in1=rb[:], op=mybir.AluOpType.min
                )

                # Batcher network over both halves, ping-pong ra <-> rb
                src, dst = ra, rb
                for sp in SORT16_STAGES:
                    f = sp["f"]
                    ni = 16 // f
                    i_full = sp["i"] == (0, ni, 1)
                    d = sp["d"]
                    di, dr = d // f, d % f
                    r_vals = list(range(*sp["r"]))
                    if r_vals[-1] + dr >= f:
                        assert all(rv + dr >= f for rv in r_vals), sp
                        di, dr = di + 1, dr - f
                    r_sl = slice(*sp["r"])
                    hi_r = slice(sp["r"][0] + dr, sp["r"][1] + dr, sp["r"][2])
                    if i_full and di == 0:
                        vs = src[:].rearrange("p (hi r bu) -> p hi r bu", r=f, bu=BU)
                        vd = dst[:].rearrange("p (hi r bu) -> p hi r bu", r=f, bu=BU)
                        lo_s = vs[:, :, r_sl, :]
                        hi_s = vs[:, :, hi_r, :]
                        nc.vector.tensor_tensor(
                            out=vd[:, :, r_sl, :], in0=lo_s, in1=hi_s,
                            op=mybir.AluOpType.min,
                        )
                        nc.vector.tensor_tensor(
                            out=vd[:, :, hi_r, :], in0=lo_s, in1=hi_s,
                            op=mybir.AluOpType.max,
                        )
                    else:
                        i_sl = slice(*sp["i"])
                        hi_i = slice(sp["i"][0] + di, sp["i"][1] + di, sp["i"][2])
                        vs = src[:].rearrange(
                            "p (hh i r bu) -> p hh i r bu", hh=2, i=ni, r=f, bu=BU
                        )
                        vd = dst[:].rearrange(
                            "p (hh i r bu) -> p hh i r bu", hh=2, i=ni, r=f, bu=BU
                        )
                        lo_s = vs[:, :, i_sl, r_sl, :]
                        hi_s = vs[:, :, hi_i, hi_r, :]
                        nc.vector.tensor_tensor(
                            out=vd[:, :, i_sl, r_sl, :], in0=lo_s, in1=hi_s,
                            op=mybir.AluOpType.min,
                        )
                        nc.vector.tensor_tensor(
                            out=vd[:, :, hi_i, hi_r, :], in0=lo_s, in1=hi_s,
                            op=mybir.AluOpType.max,
                        )
                    vks = src[:].rearrange("p (hh kk bu) -> p hh kk bu", hh=2, kk=16)
                    vkd = dst[:].rearrange("p (hh kk bu) -> p hh kk bu", hh=2, kk=16)
                    for cpsl in sp["cp"]:
                        ks = slice(*cpsl)
                        nc.scalar.copy(vkd[:, :, ks, :], vks[:, :, ks, :])
                    src, dst = dst, src

                # anti-diagonal merge of the two sorted 16-plane halves
                vk = src[:].rearrange("p (k bu) -> p k bu", k=K)
                vo = dst[:].rearrange("p (k bu) -> p k bu", k=K)
                A = vk[:, 0:16, :]
                Brev = vk[:, 31:15:-1, :]
                nc.vector.tensor_tensor(
                    out=vo[:, 0:16, :], in0=A, in1=Brev, op=mybir.AluOpType.max
                )
                nc.vector.tensor_tensor(
                    out=vk[:, 0:16, :], in0=A, in1=Brev, op=mybir.AluOpType.min
                )
                low = g_out.tile([P, BU], F32, tag="low")
                up = g_out.tile([P, BU], F32, tag="up")
                src_r = src[:].rearrange("p (k bu) -> p bu k", k=K)[:, :, 0:16]
                dst_r = dst[:].rearrange("p (k bu) -> p bu k", k=K)[:, :, 0:16]
                nc.vector.tensor_reduce(
                    out=low[:], in_=src_r, axis=mybir.AxisListType.X,
                    op=mybir.AluOpType.max,
                )
                nc.vector.tensor_reduce(
                    out=up[:], in_=dst_r, axis=mybir.AxisListType.X,
                    op=mybir.AluOpType.min,
                )
                ms = med_all[:, c * BU : (c + 1) * BU]
                nc.vector.tensor_tensor(
                    out=ms, in0=low[:], in1=up[:], op=mybir.AluOpType.add
                )
                nc.scalar.mul(ms, ms, 0.5)
                nc.sync.dma_start(out[c], ms)

    nc.compile()
    return nc


def _prep_inputs(x, neighbors, kern, num_cores=NUM_CORES, C=CHUNK):
    nrows = x.shape[0]
    HALF = nrows // 2
    total = neighbors.shape[0]
    shard = (total + num_cores - 1) // num_cores
    NCHUNK = (shard + C - 1) // C
    shard_pad = NCHUNK * C
    B = C // P
    NIDX = C * K
    IDXCOLS = NIDX // 16

    xT = np.ascontiguousarray(x.T).astype(np.float32, copy=False)
    wk = np.ascontiguousarray(kern).astype(np.float32, copy=False)

    in_maps = []
    for core in range(num_cores):
        n0 = core * shard
        nbr = np.full((shard_pad, K), nrows, dtype=np.int64)
        real = min(shard, total - n0)
        nbr[:real] = neighbors[n0 : n0 + real]
        idxarr = np.empty((NCHUNK, 2, P, IDXCOLS), dtype=np.int16)
        for c in range(NCHUNK):
            nb3 = nbr[c * C : (c + 1) * C].reshape(B, P, K)
            v = nb3.transpose(2, 0, 1).reshape(-1)  # i = ((k*B + b)*128 + p)
            lo = np.where(v < HALF, v, HALF).astype(np.int16)
            hi = np.where(v >= HALF, v - HALF, HALF).astype(np.int16)
            for j, arr in ((0, lo), (1, hi)):
                # logical index i lives at [i%16, i//16]; replicated to all
                # eight 16-partition groups (Q7 core pairs read their own)
                idxarr[c, j] = np.tile(arr.reshape(IDXCOLS, 16).T, (P // 16, 1))
        in_maps.append({"xT": xT, "wk": wk, "idx": idxarr})
    meta = dict(shard=shard, shard_pad=shard_pad, NCHUNK=NCHUNK, C=C, total=total)
    return in_maps, meta


def _unshard_output(results, meta, num_cores=NUM_CORES):
    outs = []
    for core in range(num_cores):
        o = results[core]["out"]  # [NCHUNK, P, B*U]
        NCHUNK, _, BU = o.shape
        B = BU // U
        o = (
            o.reshape(NCHUNK, P, B, U)
            .transpose(0, 2, 1, 3)
            .reshape(meta["shard_pad"], U)
        )
        outs.append(o[: meta["shard"]])
    return np.concatenate(outs, axis=0)[: meta["total"]]


_CACHE = {}


def kernel(x, neighbors, kernel):
    """Full inputs in, full output out. Shards nodes across 8 NeuronCores."""
    x = np.asarray(x, dtype=np.float32)
    neighbors_np = np.asarray(neighbors)
    kern = np.asarray(kernel, dtype=np.float32)
    assert x.shape[1] == FEAT and kern.shape == (FEAT, U)
    assert neighbors_np.shape[1] == K

    in_maps, meta = _prep_inputs(x, neighbors_np, kern)
    key = (x.shape[0], meta["shard_pad"], meta["C"])
    if key not in _CACHE:
        _CACHE[key] = build_kernel(x.shape[0], meta["shard_pad"], meta["C"])
    nc = _CACHE[key]
    res = bass_utils.run_bass_kernel_spmd(
        nc, in_maps, core_ids=list(range(NUM_CORES))
    )
    return _unshard_output(res.results, meta)

